# revision 62
# baseline (speedup 1.0000x reference)
"""Trainium2 Bass kernel for 16-head self-attention (B=2, S=2048, D=1024).

Sharding: 8 cores = 2 batches x 4 head-groups (4 heads each).  Wq/Wk/Wv are
column-split, Wo row-split (tensor parallel over heads) + data parallel over
batch.  Each core computes a partial [S, D] output (bf16); host sums the 4
partials per batch (the TP reduce) and stacks the 2 batches.

Host-side prep (layout only, no model FLOPs): q/k/v are transposed to
d-major [D, S] and cast to bf16; the mask is transposed to [keys, queries]
and stored fp8 (0/1 are exact); the weight matrices are packed bf16
(q/k/v column-blocks in one [128, 6144] tile, wo row-blocks in a separate
double-buffered [128, 2048] tile so the next execution's weight loads can
start early).

Per-core device pipeline (PE matmuls contract along SBUF partitions):
  1. Projections: vb = x @ Wv.T s-major with a constant ones column per
     head ([S, 4x(64+1)]); kT = (x @ Wk.T).T d-major [256, S]; qT likewise,
     one q-chunk at a time; all bf16, evicted on DVE.
  2. Attention per (q-chunk j, key-chunk kc): scoresT[k,q] = kT-slice.T @
     qT (two heads packed in PE rows via tile_position), exp via ACT (1/8
     scale folded) -> bf16, multiply by the resident fp8 transposed mask --
     the tile's two head-halves in parallel on the Pool and DVE engines, so
     the attention tile is ready one mask-op earlier.  Then one matmul per
     head ctx[0:65,q] += [v | 1].T @ attnT accumulates BOTH the context
     rows (0..63) and the softmax denominator (row 64) in PSUM over kc --
     the denominator rides along for free since matmul cost only scales
     with the streamed free dim.  Softmax max-subtraction is skipped:
     scores ~ N(0,1) so fp32 exp is safe, and masked entries are exactly
     zeroed by the multiply.
  3. Normalize per j: reciprocal of the denominator row -> partition
     broadcast via a K=1 float32r outer-product matmul (1 cycle/row vs 4
     for fp32) -> multiply into ctxT on PSUM eviction.  Odd heads land on
     partitions 64..127 of ctxT via a small SBUF->SBUF DMA (engines cannot
     cross partitions; the DMA ring is idle).
  4. Output projection: out-rows = ctxT.T @ woT accumulated over 2
     dk-chunks, evict bf16, DMA out.

The attention inner loop is ACT(exp)-bound, so PE instructions issued in
program order would idle ~20% there while the projections and output
projection serialize outside it.  The emission is therefore software
pipelined: k-proj (j'>=1), next-chunk q-proj, and prev-chunk out-proj are
emitted as filler packs INSIDE the attention kc loops, filling PE gaps.
All PSUM is tag-rotated: 4 banks hold per-head ctx+denom accumulators, 4
banks rotate scores/projection/out-proj/broadcast tiles.

`reps` repeats the whole body back-to-back inside one program; timing the
delta between two rep counts isolates steady-state per-execution device
time from host dispatch overhead.
"""

import sys
from contextlib import ExitStack

import numpy as np

sys.path.insert(0, "/opt/trn_rl_repo")

import concourse.bacc as bacc
import concourse.bass as bass
import concourse.mybir as mybir
import concourse.tile as tile
from concourse.bass import ds, ts

B, S, D, H = 2, 2048, 1024, 16
DK = D // H  # 64
NCORES = 8
GH = H // (NCORES // B)  # 4 heads per core
GD = GH * DK  # 256 projected dims per core

F32 = mybir.dt.float32
F32R = mybir.dt.float32r
BF16 = mybir.dt.bfloat16
FP8 = mybir.dt.float8e4

P = 128
NQ = 512  # q free-dim chunk in the attention loop
WFREE = 3 * (D // P) * GD  # 6144 packed q/k/v weight cols
WOFREE = (GD // P) * D  # 2048 packed wo cols


def build_nc(s=S, d=D, gh=GH, dk=DK, sim=False, phases=3, out_dt="bf16", reps=1):
    gd = gh * dk
    SC = s // P  # 128-row chunks (also key chunks)
    DC = d // P
    GDC = gd // P
    JC = s // NQ
    HPAIRS = gh // 2
    scale = float(1.0 / np.sqrt(dk))

    nc = bacc.Bacc("TRN2", target_bir_lowering=False, debug=sim)
    # bf16 operand: [xqT|xkT|xvT (3*d*s)] [wpk (P*WFREE)]
    WOFF = 3 * d * s
    blob = nc.dram_tensor("blob", [WOFF + P * (WFREE + WOFREE)], BF16, kind="ExternalInput")
    xT3 = [
        blob[ds(proj * d * s, d * s)].rearrange("(c p s) -> p c s", p=P, s=s)
        for proj in range(3)
    ]
    wpk = blob[ds(WOFF, P * (WFREE + WOFREE))].rearrange("(p f) -> p f", p=P)
    # fp8 operand: transposed mask [keys, q]
    mask8 = nc.dram_tensor("mask8", [s * s], FP8, kind="ExternalInput")
    maskT = mask8.rearrange("(c p q) -> p c q", p=P, q=s)
    ODT = BF16 if out_dt == "bf16" else F32
    out = nc.dram_tensor("out", [s, d], ODT, kind="ExternalOutput")
    dbg = {}
    if phases < 3:
        dbg["qT"] = nc.dram_tensor("dbg_qT", [P, JC, GDC, NQ], BF16, kind="ExternalOutput")
        dbg["kT"] = nc.dram_tensor("dbg_kT", [P, GDC, s], BF16, kind="ExternalOutput")
        dbg["vb"] = nc.dram_tensor(
            "dbg_vb", [P, SC, gh, dk + 1], BF16, kind="ExternalOutput"
        )
    if phases == 2:
        dbg["ctxT"] = nc.dram_tensor("dbg_ctxT", [P, GDC, s], BF16, kind="ExternalOutput")

    # packed weight slices: wq/wk/wv at proj*DC*gd, row-chunk kc at kc*gd;
    # wo at 3*DC*gd, dk-chunk kc at kc*d.
    def w_qkv(wt, proj, kc):
        return wt[:, ds(proj * DC * gd + kc * gd, gd)]

    def w_o(wo, kc):
        return wo[:, ds(kc * d, d)]

    with tile.TileContext(nc) as tc, ExitStack() as top:
        consts = top.enter_context(tc.tile_pool(name="consts", bufs=1))
        sb = top.enter_context(tc.tile_pool(name="sb", bufs=1))
        xpool = top.enter_context(tc.tile_pool(name="xpool", bufs=2))
        xqp = top.enter_context(tc.tile_pool(name="xqp", bufs=2))
        qp = top.enter_context(tc.tile_pool(name="qp", bufs=2))
        wop = top.enter_context(tc.tile_pool(name="wop", bufs=2))
        ctxp = top.enter_context(tc.tile_pool(name="ctxp", bufs=2))
        csp = top.enter_context(tc.tile_pool(name="csp", bufs=2))
        attnp = top.enter_context(tc.tile_pool(name="attnp", bufs=10))
        smalls = top.enter_context(tc.tile_pool(name="smalls", bufs=1))
        outp = top.enter_context(tc.tile_pool(name="outp", bufs=4))
        ps_acc = top.enter_context(tc.tile_pool(name="ps_acc", bufs=1, space="PSUM"))
        ps_sc = top.enter_context(tc.tile_pool(name="ps_sc", bufs=2, space="PSUM"))

        ones32 = consts.tile([P, dk], F32)
        nc.any.memset(ones32[:], 1.0)
        onesr = consts.tile([P, dk], F32R)
        with nc.allow_low_precision(reason="f32r ones for denom broadcast"):
            nc.vector.tensor_copy(onesr[:], ones32[:])

        for rep in range(reps):
            rn = f"_r{rep}" if reps > 1 else ""
            wt = sb.tile([P, WFREE], BF16, tag="wt", name=f"wt{rn}")
            wo = wop.tile([P, WOFREE], BF16, tag="wo", name=f"wo{rn}")
            mT = sb.tile([P, SC, s], FP8, tag="mT", name=f"mT{rn}")

            kT = sb.tile([P, GDC, s], BF16, tag="kT", name=f"kT{rn}")
            # [s, gh*(dk+1)] s-major v with a ones column per head (the ones
            # column makes each ctx matmul also accumulate the softmax denom)
            vb = sb.tile([P, SC, gh, dk + 1], BF16, tag="vb", name=f"vb{rn}")
            nc.any.memset(vb[:], 1.0)

            # phase-0 DMA schedule.  DMA transfer time occupies the issuing
            # engine's queue, so: Pool gets only early loads (idle until the
            # first mask mult), the ACT queue only loads that finish before
            # exp starts, and SP carries everything needed mid-loop.
            nwq = DC * gd
            # SP: xq0 first (q-proj starts at ~3us), then xv in quarters
            xt2 = xpool.tile([P, DC, s], BF16, tag="xt", name=f"xt2{rn}")
            xt2_src = xT3[2]
            # ACT queue: wq/wk early, then mask chunks + wo (all pre-exp)
            nc.scalar.dma_start(wt[:, ds(0, nwq)], wpk[:, ds(0, nwq)])  # wq
            nc.scalar.dma_start(wt[:, ds(1 * nwq, nwq)], wpk[:, ds(1 * nwq, nwq)])  # wk
            # Pool queue: wv, then xk (k-proj starts once it lands)
            nc.gpsimd.dma_start(wt[:, ds(2 * nwq, nwq)], wpk[:, ds(2 * nwq, nwq)])  # wv
            xt1 = xpool.tile([P, DC, s], BF16, tag="xt", name=f"xt1{rn}")
            for q4 in range(4):
                sl = ds(q4 * (s // 4), s // 4)
                nc.gpsimd.dma_start(xt1[:, :, sl], xT3[1][:, :, sl])
            for c4 in range(2):
                sl = ds(c4 * (SC // 4), SC // 4)
                nc.scalar.dma_start(mT[:, sl, :], maskT[:, sl, :])
            for c4 in range(2, 4):
                # later mask chunks ride the Pool queue after xk
                sl = ds(c4 * (SC // 4), SC // 4)
                nc.gpsimd.dma_start(mT[:, sl, :], maskT[:, sl, :])
            nc.scalar.dma_start(wo[:], wpk[:, ds(3 * nwq, GDC * d)])  # wo

            # per-j x chunks for the q projection
            def load_xq(j):
                xq = xqp.tile([P, DC, NQ], BF16, tag="xq", name=f"xq{j}{rn}")
                nc.sync.dma_start(xq[:], xT3[0][:, :, ds(j * NQ, NQ)])
                return xq

            xqs = {0: load_xq(0)}
            for q4 in range(4):
                sl = ds(q4 * (s // 4), s // 4)
                nc.sync.dma_start(xt2[:, :, sl], xt2_src[:, :, sl])

            # ====================== projection packs =======================
            def v_pack(sc):
                pp = ps_sc.tile([P, NQ], F32, tag="sc", name=f"ppv_{sc}{rn}")
                for kc in range(DC):
                    nc.tensor.matmul(
                        pp[:, :gd],
                        xt2[:, kc, ts(sc, P)],
                        w_qkv(wt, 2, kc),
                        start=(kc == 0),
                        stop=(kc == DC - 1),
                    )
                nc.vector.tensor_copy(
                    vb[:, sc, :, ds(0, dk)],
                    pp[:, :gd].rearrange("p (h k) -> p h k", k=dk),
                )

            def k_pack(j, mc):
                pp = ps_sc.tile([P, NQ], F32, tag="sc", name=f"ppk_{j}_{mc}{rn}")
                for kc in range(DC):
                    nc.tensor.matmul(
                        pp[:],
                        w_qkv(wt, 1, kc)[:, ts(mc, P)],
                        xt1[:, kc, ds(j * NQ, NQ)],
                        start=(kc == 0),
                        stop=(kc == DC - 1),
                    )
                nc.vector.tensor_copy(kT[:, mc, ds(j * NQ, NQ)], pp[:])

            qTs = {}

            def q_pack(j, mc):
                if j not in qTs:
                    qTs[j] = qp.tile([P, GDC, NQ], BF16, tag="qT", name=f"qT{j}{rn}")
                pp = ps_sc.tile([P, NQ], F32, tag="sc", name=f"ppq_{j}_{mc}{rn}")
                for kc in range(DC):
                    nc.tensor.matmul(
                        pp[:],
                        w_qkv(wt, 0, kc)[:, ts(mc, P)],
                        xqs[j][:, kc, :],
                        start=(kc == 0),
                        stop=(kc == DC - 1),
                    )
                nc.vector.tensor_copy(qTs[j][:, mc, :], pp[:])

            # ======================= out-proj packs ========================
            ctxTs = {}
            ots = {}

            def o_pack(j, scl, nj):
                if (j, scl) not in ots:
                    ots[(j, scl)] = outp.tile(
                        [P, d], ODT, tag="ot", name=f"ot_{j}_{scl}{rn}"
                    )
                ot = ots[(j, scl)]
                po = ps_sc.tile([P, NQ], F32, tag="sc", name=f"po_{j}_{scl}_{nj}{rn}")
                for kc in range(GDC):
                    nc.tensor.matmul(
                        po[:],
                        ctxTs[j][:, kc, ts(scl, P)],
                        w_o(wo, kc)[:, ds(nj * NQ, NQ)],
                        start=(kc == 0),
                        stop=(kc == GDC - 1),
                    )
                nc.vector.tensor_copy(ot[:, ds(nj * NQ, NQ)], po[:])
                if nj == d // NQ - 1:
                    nc.sync.dma_start(out[ts(j * (NQ // P) + scl, P), :], ot[:])

            # =========================== schedule ==========================
            # head: q-proj j=0 (x chunk lands first), v-proj first half
            # (fills the PE while xk streams in), k-proj j'=0, rest of v
            for mc in range(GDC):
                q_pack(0, mc)
            for sc in range(SC // 2):
                v_pack(sc)
            for mc in range(GDC):
                k_pack(0, mc)

            if phases < 3:
                # no filler interleave in debug modes: emit everything upfront
                for sc in range(SC // 2, SC):
                    v_pack(sc)
                for j in range(JC):
                    if j > 0:
                        xqs[j] = load_xq(j)
                        for mc in range(GDC):
                            k_pack(j, mc)
                        for mc in range(GDC):
                            q_pack(j, mc)
                    if phases < 2:
                        nc.sync.dma_start(dbg["qT"][:, j], qTs[j][:])
                if phases < 2:
                    nc.sync.dma_start(dbg["kT"][:], kT[:])
                    nc.sync.dma_start(dbg["vb"][:], vb[:])
                    nc.finalize() if not sim else nc.compile()
                    return nc

            ctx_tags = [f"ctx{h}" for h in range(gh)]
            atq = {}  # (j, kc) -> at tile pair
            sc_next = {j: 0 for j in range(JC)}

            def sc_block(j, kc):
                # scores + exp + mask for one (j, kc); at tiles queue in atq
                sc_next[j] = kc + 1
                qT = qTs[j]
                ats = []
                for hp in range(HPAIRS):
                    heads = (2 * hp, 2 * hp + 1)
                    sc_ps = ps_sc.tile(
                        [P, 2, NQ], F32, tag="sc", name=f"sc{hp}_{j}_{kc}{rn}"
                    )
                    for i, h in enumerate(heads):
                        mc, off = divmod(h * dk, P)
                        nc.tensor.matmul(
                            sc_ps[:, i],
                            kT[:, mc, ts(kc, P)][ds(off, dk), :],
                            qT[ds(off, dk), mc, :],
                            start=True,
                            stop=True,
                            tile_position=(off, 0),
                        )
                    at = attnp.tile(
                        [P, 2, NQ], BF16, tag="at", name=f"at{hp}_{j}_{kc}{rn}"
                    )
                    nc.scalar.activation(
                        at[:], sc_ps[:], mybir.ActivationFunctionType.Exp,
                        scale=scale,
                    )
                    # mask halves in parallel on Pool and DVE
                    engs = (nc.gpsimd, nc.vector)
                    for i, eng in enumerate(engs):
                        eng.tensor_tensor(
                            at[:, i], at[:, i], mT[:, kc, ds(j * NQ, NQ)],
                            op=mybir.AluOpType.mult,
                        )
                    ats.append(at)
                atq[(j, kc)] = ats

            def ctx_block(j, kc, ctx_ps):
                ats = atq.pop((j, kc))
                for h in range(gh):
                    hp, i = divmod(h, 2)
                    nc.tensor.matmul(
                        ctx_ps[h][ds(0, dk + 1), :],
                        vb[:, kc, h, :],
                        ats[hp][:, i],
                        start=(kc == 0),
                        stop=(kc == SC - 1),
                        skip_group_check=True,
                    )

            def bc_round(j, hp, rec, ctx_ps, ctxT, cs, bc_sb):
                # broadcast 1/denom to rows 0..63, hop through SBUF on the
                # ACT engine (Copy shares the Exp table; engines cannot read
                # two PSUM operands), then evict-multiply the pair's ctx rows
                # (DVE even heads, Pool odd ones)
                bc = ps_sc.tile([P, 2, NQ], F32, tag="sc", name=f"bc_{j}_{hp}{rn}")
                for i in range(2):
                    # f32r operands: 1 PE cycle/row instead of 4 for f32
                    nc.tensor.matmul(
                        bc[ds(0, dk), i, :],
                        onesr[ds(dk, 1), :],
                        rec[ds(dk, 1), 2 * hp + i, :],
                        start=True,
                        stop=True,
                        tile_position=(dk, 0),
                        skip_group_check=True,
                    )
                nc.scalar.copy(
                    bc_sb[ds(0, dk), ds(2 * hp, 2), :], bc[ds(0, dk), :]
                )
                for i in range(2):
                    h = 2 * hp + i
                    # Pool cannot touch PSUM, so both evict-mults go to DVE
                    dst = ctxT if h % 2 == 0 else cs
                    nc.vector.tensor_tensor(
                        dst[ds(0, dk), h // 2, :],
                        ctx_ps[h][ds(0, dk), :],
                        bc_sb[ds(0, dk), h, :],
                        op=mybir.AluOpType.mult,
                    )

            def attention(j, fillers, post=()):
                # per-head accumulators: rows 0..63 ctx, row 64 denominator
                ctx_ps = [
                    ps_acc.tile([P, NQ], F32, tag=ctx_tags[h], name=f"ctx_{h}_{j}{rn}")
                    for h in range(gh)
                ]
                n_inline = len(fillers)
                targets = [((i + 1) * SC) // (n_inline + 1) for i in range(n_inline)]
                fi = 0
                for kc in range(SC):
                    # keep the score/exp pipeline two chunks ahead of ctx
                    lead = kc + 2
                    while sc_next[j] <= lead and sc_next[j] < SC:
                        sc_block(j, sc_next[j])
                    if lead >= SC and j + 1 < JC and phases == 3 and sc_next[j + 1] == lead - SC:
                        sc_block(j + 1, lead - SC)
                    ctx_block(j, kc, ctx_ps)
                    while fi < n_inline and targets[fi] <= kc + 1:
                        fillers[fi]()
                        fi += 1
                for f in fillers[fi:]:
                    f()

                # normalize into ctxT, interleaved with the next chunk's
                # score blocks so exp never stalls across the boundary
                ctxT = ctxp.tile([P, GDC, NQ], BF16, tag="ctxT", name=f"ctxT_{j}{rn}")
                ctxTs[j] = ctxT
                cs = csp.tile([P, GDC, NQ], BF16, tag="cs", name=f"cs_{j}{rn}")
                rec = smalls.tile([P, gh, NQ], F32R, tag="rec", name=f"rec_{j}{rn}")
                with nc.allow_low_precision(reason="f32r broadcast of softmax denom"):
                    for h in range(gh):
                        nc.vector.reciprocal(
                            rec[ds(dk, 1), h, :], ctx_ps[h][ds(dk, 1), :]
                        )
                bc_sb = smalls.tile([P, gh, NQ], F32, tag="bcs", name=f"bcs_{j}{rn}")
                for hp in range(HPAIRS):
                    if j + 1 < JC and phases == 3:
                        sc_block(j + 1, sc_next[j + 1])
                    bc_round(j, hp, rec, ctx_ps, ctxT, cs, bc_sb)
                # odd heads: shift to partitions 64..127 of ctxT via DMA
                for mc in range(GDC):
                    nc.sync.dma_start(
                        ctxT[ds(dk, dk), mc, :], cs[ds(0, dk), mc, :]
                    )
                if phases == 2:
                    nc.sync.dma_start(dbg["ctxT"][:, :, ds(j * NQ, NQ)], ctxT[:])

            def xq_filler(j):
                def f():
                    xqs[j] = load_xq(j)
                return f

            # filler schedules per attention chunk.  Ordering constraints:
            # k_pack(j') must land before scores need keys 512*j' (kc=4j'),
            # v_pack(sc) before ctx kc=sc, q/xq before the next chunk starts.
            def K(jj, mc):
                return lambda: k_pack(jj, mc)

            def Q(jj, mc):
                return lambda: q_pack(jj, mc)

            def V(sc):
                return lambda: v_pack(sc)

            def O(jj, scl, nj):
                return lambda: o_pack(jj, scl, nj)

            opacks = lambda jj: [O(jj, scl, nj) for scl in range(NQ // P) for nj in range(d // NQ)]
            fillers_by_j = {
                0: [xq_filler(1), K(1, 0), K(1, 1), V(8), V(9), K(2, 0), K(2, 1),
                    V(10), V(11), K(3, 0), K(3, 1), V(12), V(13), V(14), V(15),
                    Q(1, 0), Q(1, 1), xq_filler(2)],
                1: opacks(0)[:4] + [Q(2, 0), Q(2, 1), xq_filler(3)] + opacks(0)[4:],
                2: opacks(1)[:4] + [Q(3, 0), Q(3, 1)] + opacks(1)[4:],
                3: opacks(2),
            }
            for j in range(JC):
                attention(j, fillers_by_j[j])
            for f in opacks(JC - 1):
                f()

    if sim:
        nc.compile()
    else:
        nc.finalize()
    return nc


_NC_CACHE = {}


def get_nc(**kw):
    key = tuple(sorted(kw.items()))
    if key not in _NC_CACHE:
        _NC_CACHE[key] = build_nc(**kw)
    return _NC_CACHE[key]


def _bf16(a):
    import ml_dtypes

    return np.ascontiguousarray(a.astype(ml_dtypes.bfloat16))


def _fp8(a):
    import ml_dtypes

    return np.ascontiguousarray(a.astype(ml_dtypes.float8_e4m3))


def shard_inputs(q, k, v, mask, Wq, Wk, Wv, Wo):
    q = np.asarray(q, dtype=np.float32)
    k = np.asarray(k, dtype=np.float32)
    v = np.asarray(v, dtype=np.float32)
    mask = np.asarray(mask, dtype=np.int32)
    Wq, Wk, Wv, Wo = (np.asarray(w, dtype=np.float32) for w in (Wq, Wk, Wv, Wo))

    # per-batch shared tensors, flattened to the device blob layout
    xb, mb = [], []
    for b in range(B):
        xT3 = _bf16(np.stack([q[b].T, k[b].T, v[b].T])).ravel()
        xb.append(xT3)
        mb.append(_fp8(mask[b, 0].T.astype(np.float32)).ravel())

    def pack_w(rows):
        parts = []
        for W in (Wq, Wk, Wv):
            wT = W[rows, :].T  # [D, GD]
            parts.append(wT.reshape(D // P, P, GD).transpose(1, 0, 2).reshape(P, -1))
        woT = Wo[:, rows].T  # [GD, D]
        parts.append(woT.reshape(GD // P, P, D).transpose(1, 0, 2).reshape(P, -1))
        return _bf16(np.concatenate(parts, axis=1)).ravel()

    in_maps = []
    for c in range(NCORES):
        b, g = divmod(c, NCORES // B)
        rows = slice(g * GD, (g + 1) * GD)
        in_maps.append(
            {"blob": np.concatenate([xb[b], pack_w(rows)]), "mask8": mb[b]}
        )
    return in_maps


def kernel(q, k, v, mask, Wq, Wk, Wv, Wo):
    from concourse.bass_utils import run_bass_kernel_spmd

    nc = get_nc()
    in_maps = shard_inputs(q, k, v, mask, Wq, Wk, Wv, Wo)
    res = run_bass_kernel_spmd(nc, in_maps, list(range(NCORES))).results
    out = np.zeros((B, S, D), dtype=np.float32)
    for c in range(NCORES):
        out[c // (NCORES // B)] += np.asarray(res[c]["out"]).astype(np.float32)
    return out


if __name__ == "__main__":
    nc = build_nc()
    print("built ok")
